# revision 15
# baseline (speedup 1.0000x reference)
"""Trainium2 Bass kernel for nn_CausalVAE (encoder MLP + reparam + 32-node
sequential causal decoder).

Sharding: data-parallel over batch across 8 NeuronCores (4096 rows/core),
weights replicated. On-chip layout is feature-major (features on SBUF
partitions, batch on the free dim) so L1/L2 are [K<=128, M<=128] x [K, 512]
fp32r matmuls with no on-chip transposes (inputs/outputs are transposed on
the host as part of shard/gather).

Key device-side structure (v2):
- L3 (256 -> 1 per node) runs FLIPPED: the t2 activations are the PE
  stationary operand (stride-4 column chunks [128, 128]) and the W3
  k-halves are the moving operand [128, 1], so the whole per-node output
  row costs 8 matmuls of 1 moving row each instead of 2x512 rows. The
  psum [128, 4] result lands in batch order (p*4+c = column id) and is
  evacuated by one tiny [128, 4] vector op (adding b3[i] via a broadcast
  scalar) plus one linear [128,4]->[1,512] DMA into the y-state row.
- y rows therefore carry b3 (no bias-fold, no output fixup pass) and the
  y state has no duplicate partition block.
- encoder elementwise tail (exp*eps mul, y0 add) runs on GPSIMD to keep
  DVE/ACT free for the decoder psum evacuations.
- the decoder is emitted as a software pipeline because engine queues
  execute in emission order.

Matmul precision switchable via VAE_MM_MODE: float32r (default, tf32-like,
full speed) or float32 (exact, 4x slower). L3-flip always runs fp32.
"""

import os

import numpy as np

import concourse.bass as bass
import concourse.mybir as mybir
import concourse.tile as tile
from concourse import bacc
from concourse.alu_op_type import AluOpType
from concourse.bass import ts
from concourse.bass_utils import run_bass_kernel_spmd

D = 32          # causal nodes / feature dim of y
DF = 256        # hidden dim
B = 32768       # full batch
NCORES = 8
BL = B // NCORES          # 4096 rows per core
BT = 512                  # batch tile (matmul moving free dim)
NBT = BL // BT            # 8 batch tiles per core

F32 = mybir.dt.float32
F32R = mybir.dt.float32r
AF = mybir.ActivationFunctionType

# "float32r" (fast, tf32-like) or "float32" (exact, 4x slower matmul)
MM_MODE = os.environ.get("VAE_MM_MODE", "float32r")


def _make_nc():
    rmode = MM_MODE == "float32r"

    def r(ap):
        """View an AP as float32r (matmul operands + their producers)."""
        return ap.bitcast(F32R) if rmode else ap

    nc = bacc.Bacc("TRN2", target_bir_lowering=False, debug=False)

    # ---- DRAM I/O (activations pre-transposed on host: [feat, batch]) ----
    xorT_d = nc.dram_tensor("xorT", [D, BL], F32, kind="ExternalInput")
    epsT_d = nc.dram_tensor("epsT", [D, BL], F32, kind="ExternalInput")
    We1_d = nc.dram_tensor("We1", [D, DF], F32, kind="ExternalInput")
    be1_d = nc.dram_tensor("be1", [DF], F32, kind="ExternalInput")
    We2_d = nc.dram_tensor("We2", [DF, DF], F32, kind="ExternalInput")
    be2_d = nc.dram_tensor("be2", [DF], F32, kind="ExternalInput")
    We3_d = nc.dram_tensor("We3", [DF, 2 * D], F32, kind="ExternalInput")
    be3_d = nc.dram_tensor("be3", [2 * D], F32, kind="ExternalInput")
    W1_d = nc.dram_tensor("W1", [D, D, DF], F32, kind="ExternalInput")
    b1_d = nc.dram_tensor("b1", [D, DF], F32, kind="ExternalInput")
    W2_d = nc.dram_tensor("W2", [D, DF, DF], F32, kind="ExternalInput")
    b2_d = nc.dram_tensor("b2", [D, DF], F32, kind="ExternalInput")
    W3_d = nc.dram_tensor("W3", [D, DF], F32, kind="ExternalInput")
    b3bc_d = nc.dram_tensor("b3bc", [128, D], F32, kind="ExternalInput")
    yT_dr = nc.dram_tensor("yT", [D, BL], F32, kind="ExternalOutput")
    muT_d = nc.dram_tensor("muT", [D, BL], F32, kind="ExternalOutput")
    lvT_d = nc.dram_tensor("lvT", [D, BL], F32, kind="ExternalOutput")

    with tile.TileContext(nc) as tc:
        with (
            tc.tile_pool(name="wpool", bufs=1) as wp,
            tc.tile_pool(name="actp", bufs=1) as actp,
            tc.tile_pool(name="psum", bufs=8, space="PSUM") as psp,
        ):
            # ---- persistent feature-major activations ----
            xorT = actp.tile([D, BL], F32)       # xor^T
            epsT = actp.tile([D, BL], F32)       # eps^T
            mulvT = actp.tile([2 * D, BL], F32)  # rows 0:32 mu^T, 32:64 lv^T
            yT = [actp.tile([D, BT], F32, name=f"yT{b}") for b in range(NBT)]

            # ---- inputs + encoder weights first (startup-critical) ----
            # warm the ACT func table during the input-DMA latency window
            warm = wp.tile([1, 1], F32)
            nc.vector.memset(warm[:], 0.0)
            nc.scalar.activation(warm[:], warm[:], AF.Relu)
            # first btile + We1 land first so the encoder starts immediately
            nc.sync.dma_start(out=r(xorT[:, 0:BT]), in_=r(xorT_d[:, 0:BT]))
            We1sb = wp.tile([D, DF], F32)
            nc.sync.dma_start(out=r(We1sb[:]), in_=r(We1_d[:]))
            nc.sync.dma_start(out=r(xorT[:, BT:BL]), in_=r(xorT_d[:, BT:BL]))
            for h in range(2):
                cs = ts(h, BL // 2)
                nc.sync.dma_start(out=epsT[:, cs], in_=epsT_d[:, cs])
            We2sb = wp.tile([128, 2, DF], F32)
            nc.sync.dma_start(out=r(We2sb[:]),
                              in_=r(We2_d[:].rearrange("(k p) c -> p k c", k=2)))
            We3sb = wp.tile([128, 2, 2 * D], F32)
            nc.sync.dma_start(out=r(We3sb[:]),
                              in_=r(We3_d[:].rearrange("(k p) c -> p k c", k=2)))
            be1sb = wp.tile([128, 2], F32)
            nc.sync.dma_start(out=be1sb[:], in_=be1_d[:].rearrange("(m p) -> p m", m=2))
            be2sb = wp.tile([128, 2], F32)
            nc.sync.dma_start(out=be2sb[:], in_=be2_d[:].rearrange("(m p) -> p m", m=2))
            be3sb = wp.tile([2 * D, 1], F32)
            nc.sync.dma_start(out=be3sb[:], in_=be3_d[:].unsqueeze(1))

            # ---- decoder weights (queued after startup-critical DMAs).
            # W2 is 8MB and gates decoder L2: its first node-chunk goes
            # before everything else, the rest stream in behind. ----
            W2sb = wp.tile([128, D, 2, DF], F32)
            nc.sync.dma_start(
                out=r(W2sb[:, ts(0, 8), :, :]),
                in_=r(W2_d[ts(0, 8), :, :].rearrange("i (k p) c -> p i k c", k=2)))
            # W1sb[k, i, m] = W1[i][k, m]: per-node L1 stationary [K=32, 256]
            W1sb = wp.tile([D, D, DF], F32)
            nc.sync.dma_start(out=r(W1sb[:]),
                              in_=r(W1_d[:].rearrange("i k m -> k i m")))
            # W3sb[p, i, h] = W3[i][h*128+p]: per-node L3 moving halves [128,1]
            W3sb = wp.tile([128, D, 2], F32)
            nc.sync.dma_start(out=W3sb[:],
                              in_=W3_d[:].rearrange("i (h p) -> p i h", h=2))
            b1sb = wp.tile([128, D, 2], F32)
            nc.sync.dma_start(out=b1sb[:], in_=b1_d[:].rearrange("i (m p) -> p i m", m=2))
            b2sb = wp.tile([128, D, 2], F32)
            nc.sync.dma_start(out=b2sb[:], in_=b2_d[:].rearrange("i (m p) -> p i m", m=2))
            # b3 broadcast across partitions (host-side): b3bc[p, i] = b3[i]
            b3bc = wp.tile([128, D], F32)
            nc.sync.dma_start(out=b3bc[:], in_=b3bc_d[:])

            with (
                tc.tile_pool(name="hid1", bufs=3) as h1p,
                tc.tile_pool(name="hid2", bufs=3) as h2p,
                tc.tile_pool(name="smallp", bufs=2) as smp,
            ):
                # ---- encoder, feature-major, 3-stage software pipeline ----
                enc_s1, enc_s2 = {}, {}

                def enc_front(bt):
                    bs = ts(bt, BT)
                    p1a = psp.tile([128, BT], F32, tag="ps", name="p1a")
                    nc.tensor.matmul(p1a[:], r(We1sb[:, 0:128]), r(xorT[:, bs]),
                                     start=True, stop=True)
                    p1b = psp.tile([128, BT], F32, tag="ps", name="p1b")
                    nc.tensor.matmul(p1b[:], r(We1sb[:, 128:256]), r(xorT[:, bs]),
                                     start=True, stop=True)
                    h1a = h1p.tile([128, BT], F32, tag="t1a", name="h1a")
                    nc.scalar.activation(r(h1a[:]), p1a[:], AF.Relu, bias=be1sb[:, 0:1])
                    h1b = h1p.tile([128, BT], F32, tag="t1b", name="h1b")
                    nc.vector.tensor_scalar(r(h1b[:]), p1b[:], be1sb[:, 1:2], 0.0,
                                            AluOpType.add, AluOpType.max)
                    enc_s1[bt] = (h1a, h1b)

                def enc_mid(bt):
                    h1a, h1b = enc_s1.pop(bt)
                    p2a = psp.tile([128, BT], F32, tag="ps", name="p2a")
                    nc.tensor.matmul(p2a[:], r(We2sb[:, 0, 0:128]), r(h1a[:]),
                                     start=True, stop=False)
                    nc.tensor.matmul(p2a[:], r(We2sb[:, 1, 0:128]), r(h1b[:]),
                                     start=False, stop=True)
                    p2b = psp.tile([128, BT], F32, tag="ps", name="p2b")
                    nc.tensor.matmul(p2b[:], r(We2sb[:, 0, 128:256]), r(h1a[:]),
                                     start=True, stop=False)
                    nc.tensor.matmul(p2b[:], r(We2sb[:, 1, 128:256]), r(h1b[:]),
                                     start=False, stop=True)
                    h2a = h2p.tile([128, BT], F32, tag="t2a", name="h2a")
                    nc.scalar.activation(r(h2a[:]), p2a[:], AF.Relu, bias=be2sb[:, 0:1])
                    h2b = h2p.tile([128, BT], F32, tag="t2b", name="h2b")
                    nc.vector.tensor_scalar(r(h2b[:]), p2b[:], be2sb[:, 1:2], 0.0,
                                            AluOpType.add, AluOpType.max)
                    enc_s2[bt] = (h2a, h2b)

                def enc_tail(bt):
                    bs = ts(bt, BT)
                    h2a, h2b = enc_s2.pop(bt)
                    pz = psp.tile([2 * D, BT], F32, tag="ps", name="pz")
                    nc.tensor.matmul(pz[:], r(We3sb[:, 0, :]), r(h2a[:]),
                                     start=True, stop=False)
                    nc.tensor.matmul(pz[:], r(We3sb[:, 1, :]), r(h2b[:]),
                                     start=False, stop=True)
                    nc.vector.tensor_scalar_add(mulvT[:, bs], pz[:], be3sb[:])
                    # mu / lv stream straight out (host de-transposes)
                    nc.sync.dma_start(out=muT_d[:, bs], in_=mulvT[0:D, bs])
                    nc.sync.dma_start(out=lvT_d[:, bs], in_=mulvT[D:2 * D, bs])
                    # y0 = mu + exp(lv/2) * eps; mul+add on GPSIMD (sbuf-only)
                    et = smp.tile([D, BT], F32, tag="et", name="et")
                    nc.scalar.activation(et[:], mulvT[D:2 * D, bs], AF.Exp, scale=0.5)
                    tm = smp.tile([D, BT], F32, tag="tm", name="tm")
                    nc.gpsimd.tensor_mul(tm[:], et[:], epsT[:, bs])
                    nc.gpsimd.tensor_add(r(yT[bt][:]), tm[:], mulvT[0:D, bs])

                # (decoder stage defs below; emission is unified with the
                # encoder so decoder work fills encoder-region engine gaps)

                # ---- sequential causal decoder ----
                # Software-pipelined emission: the in-order engine queues
                # preserve emission order, so interleaving stages of
                # consecutive iterations is what lets iteration k+1's L1 run
                # while k's relu/L2 are still in flight.
                NIT = D * NBT
                st1, st2, st3, st4 = {}, {}, {}, {}

                def stage_l1(k):
                    i, b = divmod(k, NBT)
                    ke = i + 1
                    if i == D - 1:
                        # rows 0..D-2 of btile b are final: stream them out now;
                        # row D-1 goes straight from its evac tile (below), so
                        # the tail never waits on a full-tile output DMA.
                        nc.sync.dma_start(out=yT_dr[0:D - 1, ts(b, BT)],
                                          in_=yT[b][0:D - 1, :])
                    p1a = psp.tile([128, BT], F32, tag="ps", name="p1a")
                    nc.tensor.matmul(p1a[:], r(W1sb[0:ke, i, 0:128]),
                                     r(yT[b][0:ke, :]), start=True, stop=True)
                    p1b = psp.tile([128, BT], F32, tag="ps", name="p1b")
                    nc.tensor.matmul(p1b[:], r(W1sb[0:ke, i, 128:256]),
                                     r(yT[b][0:ke, :]), start=True, stop=True)
                    st1[k] = (i, b, p1a, p1b)

                def stage_relu1(k):
                    i, b, p1a, p1b = st1.pop(k)
                    t1a = h1p.tile([128, BT], F32, tag="t1a", name="t1a")
                    nc.scalar.activation(r(t1a[:]), p1a[:], AF.Relu,
                                         bias=b1sb[:, i, 0:1])
                    t1b = h1p.tile([128, BT], F32, tag="t1b", name="t1b")
                    nc.vector.tensor_scalar(r(t1b[:]), p1b[:], b1sb[:, i, 1:2],
                                            0.0, AluOpType.add, AluOpType.max)
                    st2[k] = (i, b, t1a, t1b)

                def stage_l2(k):
                    i, b, t1a, t1b = st2.pop(k)
                    p2a = psp.tile([128, BT], F32, tag="ps", name="p2a")
                    nc.tensor.matmul(p2a[:], r(W2sb[:, i, 0, 0:128]), r(t1a[:]),
                                     start=True, stop=False)
                    nc.tensor.matmul(p2a[:], r(W2sb[:, i, 1, 0:128]), r(t1b[:]),
                                     start=False, stop=True)
                    p2b = psp.tile([128, BT], F32, tag="ps", name="p2b")
                    nc.tensor.matmul(p2b[:], r(W2sb[:, i, 0, 128:256]), r(t1a[:]),
                                     start=True, stop=False)
                    nc.tensor.matmul(p2b[:], r(W2sb[:, i, 1, 128:256]), r(t1b[:]),
                                     start=False, stop=True)
                    st3[k] = (i, b, p2a, p2b)

                def stage_relu2(k):
                    i, b, p2a, p2b = st3.pop(k)
                    # plain fp32 out: consumed by the fp32 flipped L3
                    t2a = h2p.tile([128, BT], F32, tag="t2a", name="t2a")
                    nc.scalar.activation(t2a[:], p2a[:], AF.Relu,
                                         bias=b2sb[:, i, 0:1])
                    t2b = h2p.tile([128, BT], F32, tag="t2b", name="t2b")
                    nc.vector.tensor_scalar(t2b[:], p2b[:], b2sb[:, i, 1:2],
                                            0.0, AluOpType.add, AluOpType.max)
                    st4[k] = (i, b, t2a, t2b)

                def stage_l3(k):
                    i, b, t2a, t2b = st4.pop(k)
                    # flipped: t2 stride-4 chunks stationary, W3 halves moving.
                    # psum (p, c) = y[i] of batch column 4p+c, so the linear
                    # [128,4]->[1,512] DMA lands in batch order.
                    py = psp.tile([128, 4], F32, tag="ps", name="py")
                    for c in range(4):
                        nc.tensor.matmul(py[:, c:c + 1], t2a[:, c::4],
                                         W3sb[:, i, 0:1], start=True, stop=False)
                        nc.tensor.matmul(py[:, c:c + 1], t2b[:, c::4],
                                         W3sb[:, i, 1:2], start=False, stop=True)
                    sb44 = smp.tile([128, 4], F32, tag="srow", name="sb44",
                                    bufs=6)
                    if k % 2 == 0:
                        nc.vector.tensor_scalar(sb44[:], py[:], b3bc[:, i:i + 1],
                                                None, AluOpType.add)
                    else:
                        nc.scalar.activation(sb44[:], py[:], AF.Identity,
                                             bias=b3bc[:, i:i + 1])
                    if i == D - 1:
                        nc.sync.dma_start(out=yT_dr[i:i + 1, ts(b, BT)],
                                          in_=sb44[:])
                    else:
                        nc.sync.dma_start(out=r(yT[b][i:i + 1, :]), in_=r(sb44[:]))

                # 4-deep pipeline: every PE stage consumes only results from
                # strictly earlier steps, so PE never waits on same-step
                # vector work.
                # Unified emission: encoder btile stages (offset by 3 so
                # tail(b) precedes decoder l1(0, b)) interleave with decoder
                # steps; W2 nodes 8..32 stream in as small pieces so the
                # per-node y-row DMAs (critical path) never queue behind a
                # multi-us weight transfer.
                w2_pieces = [slice(8 + j, 9 + j) for j in range(24)]

                for bt in range(NBT + 2):
                    if bt >= 2:
                        enc_tail(bt - 2)
                    if bt < NBT:
                        enc_front(bt)
                    if 1 <= bt < NBT + 1:
                        enc_mid(bt - 1)

                for k in range(NIT + 3):
                    if w2_pieces:
                        isl = w2_pieces.pop(0)
                        nc.sync.dma_start(
                            out=r(W2sb[:, isl, :, :]),
                            in_=r(W2_d[isl, :, :].rearrange(
                                "i (k p) c -> p i k c", k=2)))
                    if 2 <= k < NIT + 2:
                        stage_relu2(k - 2)
                    if 3 <= k:
                        stage_l3(k - 3)
                    if k < NIT:
                        stage_l1(k)
                        stage_relu1(k)
                    if 1 <= k < NIT + 1:
                        stage_l2(k - 1)


    nc.compile()
    return nc


_NC_CACHE = None


def _get_nc():
    global _NC_CACHE
    if _NC_CACHE is None:
        _NC_CACHE = _make_nc()
    return _NC_CACHE


def _run(inputs, trace=False):
    f32c = lambda a: np.ascontiguousarray(np.asarray(a), dtype=np.float32)
    xorT = f32c(inputs["xor"]).T   # [D, B]
    epsT = f32c(inputs["eps"]).T
    shared = {k: f32c(inputs[k]) for k in
              ["We1", "be1", "We2", "be2", "We3", "be3",
               "W1", "b1", "W2", "b2", "W3"]}
    shared["b3bc"] = np.ascontiguousarray(
        np.broadcast_to(f32c(inputs["b3"])[None, :], (128, D)))
    in_maps = []
    for c in range(NCORES):
        m = dict(shared)
        m["xorT"] = np.ascontiguousarray(xorT[:, c * BL:(c + 1) * BL])
        m["epsT"] = np.ascontiguousarray(epsT[:, c * BL:(c + 1) * BL])
        in_maps.append(m)
    nc = _get_nc()
    res = run_bass_kernel_spmd(nc, in_maps, core_ids=list(range(NCORES)),
                               trace=trace)
    gather = lambda nm: np.ascontiguousarray(
        np.concatenate([r[nm] for r in res.results], axis=1).T)
    y = gather("yT")
    mu = gather("muT")
    lv = gather("lvT")
    return (y, mu, lv, y), res


def kernel(**inputs):
    out, _ = _run(inputs)
    return out


# revision 16
# speedup vs baseline: 1.0115x; 1.0115x over previous
"""Trainium2 Bass kernel for nn_CausalVAE (encoder MLP + reparam + 32-node
sequential causal decoder).

Sharding: data-parallel over batch across 8 NeuronCores (4096 rows/core),
weights replicated. On-chip layout is feature-major (features on SBUF
partitions, batch on the free dim) so L1/L2 are [K<=128, M<=128] x [K, 512]
fp32r matmuls with no on-chip transposes (inputs/outputs are transposed on
the host as part of shard/gather).

Key device-side structure (v2):
- L3 (256 -> 1 per node) runs FLIPPED: the t2 activations are the PE
  stationary operand (stride-4 column chunks [128, 128]) and the W3
  k-halves are the moving operand [128, 1], so the whole per-node output
  row costs 8 matmuls of 1 moving row each instead of 2x512 rows. The
  psum [128, 4] result lands in batch order (p*4+c = column id) and is
  evacuated by one tiny [128, 4] vector op (adding b3[i] via a broadcast
  scalar) plus one linear [128,4]->[1,512] DMA into the y-state row.
- y rows therefore carry b3 (no bias-fold, no output fixup pass) and the
  y state has no duplicate partition block.
- encoder elementwise tail (exp*eps mul, y0 add) runs on GPSIMD to keep
  DVE/ACT free for the decoder psum evacuations.
- the decoder is emitted as a software pipeline because engine queues
  execute in emission order.

Matmul precision switchable via VAE_MM_MODE: float32r (default, tf32-like,
full speed) or float32 (exact, 4x slower). L3-flip always runs fp32.
"""

import os

import numpy as np

import concourse.bass as bass
import concourse.mybir as mybir
import concourse.tile as tile
from concourse import bacc
from concourse.alu_op_type import AluOpType
from concourse.bass import ts
from concourse.bass_utils import run_bass_kernel_spmd

D = 32          # causal nodes / feature dim of y
DF = 256        # hidden dim
B = 32768       # full batch
NCORES = 8
BL = B // NCORES          # 4096 rows per core
BT = 512                  # batch tile (matmul moving free dim)
NBT = BL // BT            # 8 batch tiles per core

F32 = mybir.dt.float32
F32R = mybir.dt.float32r
AF = mybir.ActivationFunctionType

# "float32r" (fast, tf32-like) or "float32" (exact, 4x slower matmul)
MM_MODE = os.environ.get("VAE_MM_MODE", "float32r")


def _make_nc():
    rmode = MM_MODE == "float32r"

    def r(ap):
        """View an AP as float32r (matmul operands + their producers)."""
        return ap.bitcast(F32R) if rmode else ap

    nc = bacc.Bacc("TRN2", target_bir_lowering=False, debug=False)

    # ---- DRAM I/O (activations pre-transposed on host: [feat, batch]) ----
    xorT_d = nc.dram_tensor("xorT", [D, BL], F32, kind="ExternalInput")
    epsT_d = nc.dram_tensor("epsT", [D, BL], F32, kind="ExternalInput")
    We1_d = nc.dram_tensor("We1", [D, DF], F32, kind="ExternalInput")
    be1_d = nc.dram_tensor("be1", [DF], F32, kind="ExternalInput")
    We2_d = nc.dram_tensor("We2", [DF, DF], F32, kind="ExternalInput")
    be2_d = nc.dram_tensor("be2", [DF], F32, kind="ExternalInput")
    We3_d = nc.dram_tensor("We3", [DF, 2 * D], F32, kind="ExternalInput")
    be3_d = nc.dram_tensor("be3", [2 * D], F32, kind="ExternalInput")
    W1_d = nc.dram_tensor("W1", [D, D, DF], F32, kind="ExternalInput")
    b1_d = nc.dram_tensor("b1", [D, DF], F32, kind="ExternalInput")
    W2_d = nc.dram_tensor("W2", [D, DF, DF], F32, kind="ExternalInput")
    b2_d = nc.dram_tensor("b2", [D, DF], F32, kind="ExternalInput")
    W3_d = nc.dram_tensor("W3", [D, DF], F32, kind="ExternalInput")
    b3bc_d = nc.dram_tensor("b3bc", [128, D], F32, kind="ExternalInput")
    yT_dr = nc.dram_tensor("yT", [D, BL], F32, kind="ExternalOutput")
    muT_d = nc.dram_tensor("muT", [D, BL], F32, kind="ExternalOutput")
    lvT_d = nc.dram_tensor("lvT", [D, BL], F32, kind="ExternalOutput")

    with tile.TileContext(nc) as tc:
        with (
            tc.tile_pool(name="wpool", bufs=1) as wp,
            tc.tile_pool(name="actp", bufs=1) as actp,
            tc.tile_pool(name="psum", bufs=8, space="PSUM") as psp,
        ):
            # ---- persistent feature-major activations ----
            xorT = actp.tile([D, BL], F32)       # xor^T
            epsT = actp.tile([D, BL], F32)       # eps^T
            mulvT = actp.tile([2 * D, BL], F32)  # rows 0:32 mu^T, 32:64 lv^T
            yT = [actp.tile([D, BT], F32, name=f"yT{b}") for b in range(NBT)]

            # ---- inputs + encoder weights first (startup-critical) ----
            # warm the ACT func table during the input-DMA latency window
            warm = wp.tile([1, 1], F32)
            nc.vector.memset(warm[:], 0.0)
            nc.scalar.activation(warm[:], warm[:], AF.Relu)
            # first btile + We1 land first so the encoder starts immediately
            nc.sync.dma_start(out=r(xorT[:, 0:BT]), in_=r(xorT_d[:, 0:BT]))
            We1sb = wp.tile([D, DF], F32)
            nc.sync.dma_start(out=r(We1sb[:]), in_=r(We1_d[:]))
            nc.sync.dma_start(out=r(xorT[:, BT:BL]), in_=r(xorT_d[:, BT:BL]))
            for h in range(2):
                cs = ts(h, BL // 2)
                nc.sync.dma_start(out=epsT[:, cs], in_=epsT_d[:, cs])
            We2sb = wp.tile([128, 2, DF], F32)
            nc.sync.dma_start(out=r(We2sb[:]),
                              in_=r(We2_d[:].rearrange("(k p) c -> p k c", k=2)))
            We3sb = wp.tile([128, 2, 2 * D], F32)
            nc.sync.dma_start(out=r(We3sb[:]),
                              in_=r(We3_d[:].rearrange("(k p) c -> p k c", k=2)))
            be1sb = wp.tile([128, 2], F32)
            nc.sync.dma_start(out=be1sb[:], in_=be1_d[:].rearrange("(m p) -> p m", m=2))
            be2sb = wp.tile([128, 2], F32)
            nc.sync.dma_start(out=be2sb[:], in_=be2_d[:].rearrange("(m p) -> p m", m=2))
            be3sb = wp.tile([2 * D, 1], F32)
            nc.sync.dma_start(out=be3sb[:], in_=be3_d[:].unsqueeze(1))

            # ---- decoder weights (queued after startup-critical DMAs).
            # W2 is 8MB and gates decoder L2: its first node-chunk goes
            # before everything else, the rest stream in behind. ----
            W2sb = wp.tile([128, D, 2, DF], F32)
            nc.sync.dma_start(
                out=r(W2sb[:, ts(0, 8), :, :]),
                in_=r(W2_d[ts(0, 8), :, :].rearrange("i (k p) c -> p i k c", k=2)))
            # W1sb[k, i, m] = W1[i][k, m]: per-node L1 stationary [K=32, 256]
            W1sb = wp.tile([D, D, DF], F32)
            nc.sync.dma_start(out=r(W1sb[:]),
                              in_=r(W1_d[:].rearrange("i k m -> k i m")))
            # W3sb[p, i, h] = W3[i][h*128+p]: per-node L3 moving halves [128,1]
            W3sb = wp.tile([128, D, 2], F32)
            nc.sync.dma_start(out=W3sb[:],
                              in_=W3_d[:].rearrange("i (h p) -> p i h", h=2))
            b1sb = wp.tile([128, D, 2], F32)
            nc.sync.dma_start(out=b1sb[:], in_=b1_d[:].rearrange("i (m p) -> p i m", m=2))
            b2sb = wp.tile([128, D, 2], F32)
            nc.sync.dma_start(out=b2sb[:], in_=b2_d[:].rearrange("i (m p) -> p i m", m=2))
            # b3 broadcast across partitions (host-side): b3bc[p, i] = b3[i]
            b3bc = wp.tile([128, D], F32)
            nc.sync.dma_start(out=b3bc[:], in_=b3bc_d[:])

            with (
                tc.tile_pool(name="hid1", bufs=3) as h1p,
                tc.tile_pool(name="hid2", bufs=3) as h2p,
                tc.tile_pool(name="smallp", bufs=2) as smp,
            ):
                # ---- encoder, feature-major, 3-stage software pipeline ----
                enc_s1, enc_s2 = {}, {}

                def enc_front(bt):
                    bs = ts(bt, BT)
                    p1a = psp.tile([128, BT], F32, tag="ps", name="p1a")
                    nc.tensor.matmul(p1a[:], r(We1sb[:, 0:128]), r(xorT[:, bs]),
                                     start=True, stop=True)
                    p1b = psp.tile([128, BT], F32, tag="ps", name="p1b")
                    nc.tensor.matmul(p1b[:], r(We1sb[:, 128:256]), r(xorT[:, bs]),
                                     start=True, stop=True)
                    h1a = h1p.tile([128, BT], F32, tag="t1a", name="h1a")
                    nc.scalar.activation(r(h1a[:]), p1a[:], AF.Relu, bias=be1sb[:, 0:1])
                    h1b = h1p.tile([128, BT], F32, tag="t1b", name="h1b")
                    nc.vector.tensor_scalar(r(h1b[:]), p1b[:], be1sb[:, 1:2], 0.0,
                                            AluOpType.add, AluOpType.max)
                    enc_s1[bt] = (h1a, h1b)

                def enc_mid(bt):
                    h1a, h1b = enc_s1.pop(bt)
                    p2a = psp.tile([128, BT], F32, tag="ps", name="p2a")
                    nc.tensor.matmul(p2a[:], r(We2sb[:, 0, 0:128]), r(h1a[:]),
                                     start=True, stop=False)
                    nc.tensor.matmul(p2a[:], r(We2sb[:, 1, 0:128]), r(h1b[:]),
                                     start=False, stop=True)
                    p2b = psp.tile([128, BT], F32, tag="ps", name="p2b")
                    nc.tensor.matmul(p2b[:], r(We2sb[:, 0, 128:256]), r(h1a[:]),
                                     start=True, stop=False)
                    nc.tensor.matmul(p2b[:], r(We2sb[:, 1, 128:256]), r(h1b[:]),
                                     start=False, stop=True)
                    h2a = h2p.tile([128, BT], F32, tag="t2a", name="h2a")
                    nc.scalar.activation(r(h2a[:]), p2a[:], AF.Relu, bias=be2sb[:, 0:1])
                    h2b = h2p.tile([128, BT], F32, tag="t2b", name="h2b")
                    nc.vector.tensor_scalar(r(h2b[:]), p2b[:], be2sb[:, 1:2], 0.0,
                                            AluOpType.add, AluOpType.max)
                    enc_s2[bt] = (h2a, h2b)

                def enc_tail(bt):
                    bs = ts(bt, BT)
                    h2a, h2b = enc_s2.pop(bt)
                    pz = psp.tile([2 * D, BT], F32, tag="ps", name="pz")
                    nc.tensor.matmul(pz[:], r(We3sb[:, 0, :]), r(h2a[:]),
                                     start=True, stop=False)
                    nc.tensor.matmul(pz[:], r(We3sb[:, 1, :]), r(h2b[:]),
                                     start=False, stop=True)
                    nc.vector.tensor_scalar_add(mulvT[:, bs], pz[:], be3sb[:])
                    # mu / lv stream straight out (host de-transposes)
                    nc.sync.dma_start(out=muT_d[:, bs], in_=mulvT[0:D, bs])
                    nc.sync.dma_start(out=lvT_d[:, bs], in_=mulvT[D:2 * D, bs])
                    # y0 = mu + exp(lv/2) * eps; mul+add on GPSIMD (sbuf-only)
                    et = smp.tile([D, BT], F32, tag="et", name="et")
                    nc.scalar.activation(et[:], mulvT[D:2 * D, bs], AF.Exp, scale=0.5)
                    tm = smp.tile([D, BT], F32, tag="tm", name="tm")
                    nc.gpsimd.tensor_mul(tm[:], et[:], epsT[:, bs])
                    nc.gpsimd.tensor_add(r(yT[bt][:]), tm[:], mulvT[0:D, bs])

                # (decoder stage defs below; emission is unified with the
                # encoder so decoder work fills encoder-region engine gaps)

                # ---- sequential causal decoder ----
                # Software-pipelined emission: the in-order engine queues
                # preserve emission order, so interleaving stages of
                # consecutive iterations is what lets iteration k+1's L1 run
                # while k's relu/L2 are still in flight.
                NIT = D * NBT
                st1, st2, st3, st4 = {}, {}, {}, {}

                def stage_l1(k):
                    i, b = divmod(k, NBT)
                    ke = i + 1
                    if i == D - 1:
                        # rows 0..D-2 of btile b are final: stream them out now;
                        # row D-1 goes straight from its evac tile (below), so
                        # the tail never waits on a full-tile output DMA.
                        nc.sync.dma_start(out=yT_dr[0:D - 1, ts(b, BT)],
                                          in_=yT[b][0:D - 1, :])
                    p1a = psp.tile([128, BT], F32, tag="ps", name="p1a")
                    nc.tensor.matmul(p1a[:], r(W1sb[0:ke, i, 0:128]),
                                     r(yT[b][0:ke, :]), start=True, stop=True)
                    p1b = psp.tile([128, BT], F32, tag="ps", name="p1b")
                    nc.tensor.matmul(p1b[:], r(W1sb[0:ke, i, 128:256]),
                                     r(yT[b][0:ke, :]), start=True, stop=True)
                    st1[k] = (i, b, p1a, p1b)

                def stage_relu1(k):
                    i, b, p1a, p1b = st1.pop(k)
                    t1a = h1p.tile([128, BT], F32, tag="t1a", name="t1a")
                    nc.scalar.activation(r(t1a[:]), p1a[:], AF.Relu,
                                         bias=b1sb[:, i, 0:1])
                    t1b = h1p.tile([128, BT], F32, tag="t1b", name="t1b")
                    nc.vector.tensor_scalar(r(t1b[:]), p1b[:], b1sb[:, i, 1:2],
                                            0.0, AluOpType.add, AluOpType.max)
                    st2[k] = (i, b, t1a, t1b)

                def stage_l2(k):
                    i, b, t1a, t1b = st2.pop(k)
                    p2a = psp.tile([128, BT], F32, tag="ps", name="p2a")
                    nc.tensor.matmul(p2a[:], r(W2sb[:, i, 0, 0:128]), r(t1a[:]),
                                     start=True, stop=False)
                    nc.tensor.matmul(p2a[:], r(W2sb[:, i, 1, 0:128]), r(t1b[:]),
                                     start=False, stop=True)
                    p2b = psp.tile([128, BT], F32, tag="ps", name="p2b")
                    nc.tensor.matmul(p2b[:], r(W2sb[:, i, 0, 128:256]), r(t1a[:]),
                                     start=True, stop=False)
                    nc.tensor.matmul(p2b[:], r(W2sb[:, i, 1, 128:256]), r(t1b[:]),
                                     start=False, stop=True)
                    st3[k] = (i, b, p2a, p2b)

                def stage_relu2(k):
                    i, b, p2a, p2b = st3.pop(k)
                    # plain fp32 out: consumed by the fp32 flipped L3
                    t2a = h2p.tile([128, BT], F32, tag="t2a", name="t2a")
                    nc.scalar.activation(t2a[:], p2a[:], AF.Relu,
                                         bias=b2sb[:, i, 0:1])
                    t2b = h2p.tile([128, BT], F32, tag="t2b", name="t2b")
                    nc.vector.tensor_scalar(t2b[:], p2b[:], b2sb[:, i, 1:2],
                                            0.0, AluOpType.add, AluOpType.max)
                    st4[k] = (i, b, t2a, t2b)

                def stage_l3(k):
                    i, b, t2a, t2b = st4.pop(k)
                    # flipped: t2 stride-4 chunks stationary, W3 halves moving.
                    # psum (p, c) = y[i] of batch column 4p+c, so the linear
                    # [128,4]->[1,512] DMA lands in batch order.
                    py = psp.tile([128, 4], F32, tag="ps", name="py")
                    for c in range(4):
                        nc.tensor.matmul(py[:, c:c + 1], t2a[:, c::4],
                                         W3sb[:, i, 0:1], start=True, stop=False)
                        nc.tensor.matmul(py[:, c:c + 1], t2b[:, c::4],
                                         W3sb[:, i, 1:2], start=False, stop=True)
                    sb44 = smp.tile([128, 4], F32, tag="srow", name="sb44",
                                    bufs=6)
                    if k % 2 == 0:
                        nc.vector.tensor_scalar(sb44[:], py[:], b3bc[:, i:i + 1],
                                                None, AluOpType.add)
                    else:
                        nc.scalar.activation(sb44[:], py[:], AF.Identity,
                                             bias=b3bc[:, i:i + 1])
                    if i == D - 1:
                        nc.sync.dma_start(out=yT_dr[i:i + 1, ts(b, BT)],
                                          in_=sb44[:])
                    else:
                        nc.sync.dma_start(out=r(yT[b][i:i + 1, :]), in_=r(sb44[:]))

                # 4-deep pipeline: every PE stage consumes only results from
                # strictly earlier steps, so PE never waits on same-step
                # vector work.
                # Unified emission: encoder btile stages (offset by 3 so
                # tail(b) precedes decoder l1(0, b)) interleave with decoder
                # steps; W2 nodes 8..32 stream in as small pieces so the
                # per-node y-row DMAs (critical path) never queue behind a
                # multi-us weight transfer.
                w2_pieces = [slice(8 + j, 9 + j) for j in range(24)]

                for bt in range(NBT + 2):
                    if bt >= 2:
                        enc_tail(bt - 2)
                    if bt < NBT:
                        enc_front(bt)
                    if 1 <= bt < NBT + 1:
                        enc_mid(bt - 1)

                for k in range(NIT + 3):
                    if k % 2 == 0 and w2_pieces:
                        isl = w2_pieces.pop(0)
                        nc.sync.dma_start(
                            out=r(W2sb[:, isl, :, :]),
                            in_=r(W2_d[isl, :, :].rearrange(
                                "i (k p) c -> p i k c", k=2)))
                    if 2 <= k < NIT + 2:
                        stage_relu2(k - 2)
                    if 3 <= k:
                        stage_l3(k - 3)
                    if k < NIT:
                        stage_l1(k)
                        stage_relu1(k)
                    if 1 <= k < NIT + 1:
                        stage_l2(k - 1)


    nc.compile()
    return nc


_NC_CACHE = None


def _get_nc():
    global _NC_CACHE
    if _NC_CACHE is None:
        _NC_CACHE = _make_nc()
    return _NC_CACHE


def _run(inputs, trace=False):
    f32c = lambda a: np.ascontiguousarray(np.asarray(a), dtype=np.float32)
    xorT = f32c(inputs["xor"]).T   # [D, B]
    epsT = f32c(inputs["eps"]).T
    shared = {k: f32c(inputs[k]) for k in
              ["We1", "be1", "We2", "be2", "We3", "be3",
               "W1", "b1", "W2", "b2", "W3"]}
    shared["b3bc"] = np.ascontiguousarray(
        np.broadcast_to(f32c(inputs["b3"])[None, :], (128, D)))
    in_maps = []
    for c in range(NCORES):
        m = dict(shared)
        m["xorT"] = np.ascontiguousarray(xorT[:, c * BL:(c + 1) * BL])
        m["epsT"] = np.ascontiguousarray(epsT[:, c * BL:(c + 1) * BL])
        in_maps.append(m)
    nc = _get_nc()
    res = run_bass_kernel_spmd(nc, in_maps, core_ids=list(range(NCORES)),
                               trace=trace)
    gather = lambda nm: np.ascontiguousarray(
        np.concatenate([r[nm] for r in res.results], axis=1).T)
    y = gather("yT")
    mu = gather("muT")
    lv = gather("lvT")
    return (y, mu, lv, y), res


def kernel(**inputs):
    out, _ = _run(inputs)
    return out


# revision 17
# speedup vs baseline: 1.0221x; 1.0105x over previous
"""Trainium2 Bass kernel for nn_CausalVAE (encoder MLP + reparam + 32-node
sequential causal decoder).

Sharding: data-parallel over batch across 8 NeuronCores (4096 rows/core),
weights replicated. On-chip layout is feature-major (features on SBUF
partitions, batch on the free dim) so L1/L2 are [K<=128, M<=128] x [K, 512]
fp32r matmuls with no on-chip transposes (inputs/outputs are transposed on
the host as part of shard/gather).

Key device-side structure (v2):
- L3 (256 -> 1 per node) runs FLIPPED: the t2 activations are the PE
  stationary operand (stride-4 column chunks [128, 128]) and the W3
  k-halves are the moving operand [128, 1], so the whole per-node output
  row costs 8 matmuls of 1 moving row each instead of 2x512 rows. The
  psum [128, 4] result lands in batch order (p*4+c = column id) and is
  evacuated by one tiny [128, 4] vector op (adding b3[i] via a broadcast
  scalar) plus one linear [128,4]->[1,512] DMA into the y-state row.
- y rows therefore carry b3 (no bias-fold, no output fixup pass) and the
  y state has no duplicate partition block.
- encoder elementwise tail (exp*eps mul, y0 add) runs on GPSIMD to keep
  DVE/ACT free for the decoder psum evacuations.
- the decoder is emitted as a software pipeline because engine queues
  execute in emission order.

Matmul precision switchable via VAE_MM_MODE: float32r (default, tf32-like,
full speed) or float32 (exact, 4x slower). L3-flip always runs fp32.
"""

import os

import numpy as np

import concourse.bass as bass
import concourse.mybir as mybir
import concourse.tile as tile
from concourse import bacc
from concourse.alu_op_type import AluOpType
from concourse.bass import ts
from concourse.bass_utils import run_bass_kernel_spmd

D = 32          # causal nodes / feature dim of y
DF = 256        # hidden dim
B = 32768       # full batch
NCORES = 8
BL = B // NCORES          # 4096 rows per core
BT = 512                  # batch tile (matmul moving free dim)
NBT = BL // BT            # 8 batch tiles per core

F32 = mybir.dt.float32
F32R = mybir.dt.float32r
AF = mybir.ActivationFunctionType

# "float32r" (fast, tf32-like) or "float32" (exact, 4x slower matmul)
MM_MODE = os.environ.get("VAE_MM_MODE", "float32r")


def _make_nc():
    rmode = MM_MODE == "float32r"

    def r(ap):
        """View an AP as float32r (matmul operands + their producers)."""
        return ap.bitcast(F32R) if rmode else ap

    nc = bacc.Bacc("TRN2", target_bir_lowering=False, debug=False)

    # ---- DRAM I/O (activations pre-transposed on host: [feat, batch]) ----
    xorT_d = nc.dram_tensor("xorT", [D, BL], F32, kind="ExternalInput")
    epsT_d = nc.dram_tensor("epsT", [D, BL], F32, kind="ExternalInput")
    We1_d = nc.dram_tensor("We1", [D, DF], F32, kind="ExternalInput")
    be1_d = nc.dram_tensor("be1", [DF], F32, kind="ExternalInput")
    We2_d = nc.dram_tensor("We2", [DF, DF], F32, kind="ExternalInput")
    be2_d = nc.dram_tensor("be2", [DF], F32, kind="ExternalInput")
    We3_d = nc.dram_tensor("We3", [DF, 2 * D], F32, kind="ExternalInput")
    be3_d = nc.dram_tensor("be3", [2 * D], F32, kind="ExternalInput")
    W1_d = nc.dram_tensor("W1", [D, D, DF], F32, kind="ExternalInput")
    b1_d = nc.dram_tensor("b1", [D, DF], F32, kind="ExternalInput")
    W2_d = nc.dram_tensor("W2", [D, DF, DF], F32, kind="ExternalInput")
    b2_d = nc.dram_tensor("b2", [D, DF], F32, kind="ExternalInput")
    W3_d = nc.dram_tensor("W3", [D, DF], F32, kind="ExternalInput")
    b3bc_d = nc.dram_tensor("b3bc", [128, D], F32, kind="ExternalInput")
    yT_dr = nc.dram_tensor("yT", [D, BL], F32, kind="ExternalOutput")
    muT_d = nc.dram_tensor("muT", [D, BL], F32, kind="ExternalOutput")
    lvT_d = nc.dram_tensor("lvT", [D, BL], F32, kind="ExternalOutput")

    with tile.TileContext(nc) as tc:
        with (
            tc.tile_pool(name="wpool", bufs=1) as wp,
            tc.tile_pool(name="actp", bufs=1) as actp,
            tc.tile_pool(name="psum", bufs=8, space="PSUM") as psp,
        ):
            # ---- persistent feature-major activations ----
            xorT = actp.tile([D, BL], F32)       # xor^T
            epsT = actp.tile([D, BL], F32)       # eps^T
            mulvT = actp.tile([2 * D, BL], F32)  # rows 0:32 mu^T, 32:64 lv^T
            yT = [actp.tile([D, BT], F32, name=f"yT{b}") for b in range(NBT)]

            # ---- inputs + encoder weights first (startup-critical) ----
            # warm the ACT func table during the input-DMA latency window
            warm = wp.tile([1, 1], F32)
            nc.vector.memset(warm[:], 0.0)
            nc.scalar.activation(warm[:], warm[:], AF.Relu)
            # first btile + We1 land first so the encoder starts immediately
            nc.sync.dma_start(out=r(xorT[:, 0:BT]), in_=r(xorT_d[:, 0:BT]))
            We1sb = wp.tile([D, DF], F32)
            nc.sync.dma_start(out=r(We1sb[:]), in_=r(We1_d[:]))
            nc.sync.dma_start(out=r(xorT[:, BT:BL]), in_=r(xorT_d[:, BT:BL]))
            for h in range(2):
                cs = ts(h, BL // 2)
                nc.sync.dma_start(out=epsT[:, cs], in_=epsT_d[:, cs])
            We2sb = wp.tile([128, 2, DF], F32)
            nc.sync.dma_start(out=r(We2sb[:]),
                              in_=r(We2_d[:].rearrange("(k p) c -> p k c", k=2)))
            We3sb = wp.tile([128, 2, 2 * D], F32)
            nc.sync.dma_start(out=r(We3sb[:]),
                              in_=r(We3_d[:].rearrange("(k p) c -> p k c", k=2)))
            be1sb = wp.tile([128, 2], F32)
            nc.sync.dma_start(out=be1sb[:], in_=be1_d[:].rearrange("(m p) -> p m", m=2))
            be2sb = wp.tile([128, 2], F32)
            nc.sync.dma_start(out=be2sb[:], in_=be2_d[:].rearrange("(m p) -> p m", m=2))
            be3sb = wp.tile([2 * D, 1], F32)
            nc.sync.dma_start(out=be3sb[:], in_=be3_d[:].unsqueeze(1))

            # ---- decoder weights (queued after startup-critical DMAs).
            # W2 is 8MB and gates decoder L2: its first node-chunk goes
            # before everything else, the rest stream in behind. ----
            W2sb = wp.tile([128, D, 2, DF], F32)
            nc.sync.dma_start(
                out=r(W2sb[:, ts(0, 8), :, :]),
                in_=r(W2_d[ts(0, 8), :, :].rearrange("i (k p) c -> p i k c", k=2)))
            # W1sb[k, i, m] = W1[i][k, m]: per-node L1 stationary [K=32, 256]
            W1sb = wp.tile([D, D, DF], F32)
            nc.sync.dma_start(out=r(W1sb[:]),
                              in_=r(W1_d[:].rearrange("i k m -> k i m")))
            # W3sb[p, i, h] = W3[i][h*128+p]: per-node L3 moving halves [128,1]
            W3sb = wp.tile([128, D, 2], F32)
            nc.sync.dma_start(out=W3sb[:],
                              in_=W3_d[:].rearrange("i (h p) -> p i h", h=2))
            b1sb = wp.tile([128, D, 2], F32)
            nc.sync.dma_start(out=b1sb[:], in_=b1_d[:].rearrange("i (m p) -> p i m", m=2))
            b2sb = wp.tile([128, D, 2], F32)
            nc.sync.dma_start(out=b2sb[:], in_=b2_d[:].rearrange("i (m p) -> p i m", m=2))
            # b3 broadcast across partitions (host-side): b3bc[p, i] = b3[i]
            b3bc = wp.tile([128, D], F32)
            nc.sync.dma_start(out=b3bc[:], in_=b3bc_d[:])

            with (
                tc.tile_pool(name="hid1", bufs=3) as h1p,
                tc.tile_pool(name="hid2", bufs=3) as h2p,
                tc.tile_pool(name="smallp", bufs=2) as smp,
            ):
                # ---- encoder, feature-major, 3-stage software pipeline ----
                enc_s1, enc_s2 = {}, {}

                def enc_front(bt):
                    bs = ts(bt, BT)
                    p1a = psp.tile([128, BT], F32, tag="ps", name="p1a")
                    nc.tensor.matmul(p1a[:], r(We1sb[:, 0:128]), r(xorT[:, bs]),
                                     start=True, stop=True)
                    p1b = psp.tile([128, BT], F32, tag="ps", name="p1b")
                    nc.tensor.matmul(p1b[:], r(We1sb[:, 128:256]), r(xorT[:, bs]),
                                     start=True, stop=True)
                    h1a = h1p.tile([128, BT], F32, tag="t1a", name="h1a")
                    nc.scalar.activation(r(h1a[:]), p1a[:], AF.Relu, bias=be1sb[:, 0:1])
                    h1b = h1p.tile([128, BT], F32, tag="t1b", name="h1b")
                    nc.vector.tensor_scalar(r(h1b[:]), p1b[:], be1sb[:, 1:2], 0.0,
                                            AluOpType.add, AluOpType.max)
                    enc_s1[bt] = (h1a, h1b)

                def enc_mid(bt):
                    h1a, h1b = enc_s1.pop(bt)
                    p2a = psp.tile([128, BT], F32, tag="ps", name="p2a")
                    nc.tensor.matmul(p2a[:], r(We2sb[:, 0, 0:128]), r(h1a[:]),
                                     start=True, stop=False)
                    nc.tensor.matmul(p2a[:], r(We2sb[:, 1, 0:128]), r(h1b[:]),
                                     start=False, stop=True)
                    p2b = psp.tile([128, BT], F32, tag="ps", name="p2b")
                    nc.tensor.matmul(p2b[:], r(We2sb[:, 0, 128:256]), r(h1a[:]),
                                     start=True, stop=False)
                    nc.tensor.matmul(p2b[:], r(We2sb[:, 1, 128:256]), r(h1b[:]),
                                     start=False, stop=True)
                    h2a = h2p.tile([128, BT], F32, tag="t2a", name="h2a")
                    nc.scalar.activation(r(h2a[:]), p2a[:], AF.Relu, bias=be2sb[:, 0:1])
                    h2b = h2p.tile([128, BT], F32, tag="t2b", name="h2b")
                    nc.vector.tensor_scalar(r(h2b[:]), p2b[:], be2sb[:, 1:2], 0.0,
                                            AluOpType.add, AluOpType.max)
                    enc_s2[bt] = (h2a, h2b)

                def enc_tail(bt):
                    bs = ts(bt, BT)
                    h2a, h2b = enc_s2.pop(bt)
                    pz = psp.tile([2 * D, BT], F32, tag="ps", name="pz")
                    nc.tensor.matmul(pz[:], r(We3sb[:, 0, :]), r(h2a[:]),
                                     start=True, stop=False)
                    nc.tensor.matmul(pz[:], r(We3sb[:, 1, :]), r(h2b[:]),
                                     start=False, stop=True)
                    nc.vector.tensor_scalar_add(mulvT[:, bs], pz[:], be3sb[:])
                    # mu / lv stream straight out (host de-transposes)
                    nc.sync.dma_start(out=muT_d[:, bs], in_=mulvT[0:D, bs])
                    nc.sync.dma_start(out=lvT_d[:, bs], in_=mulvT[D:2 * D, bs])
                    # y0 = mu + exp(lv/2) * eps; mul+add on GPSIMD (sbuf-only)
                    et = smp.tile([D, BT], F32, tag="et", name="et")
                    nc.scalar.activation(et[:], mulvT[D:2 * D, bs], AF.Exp, scale=0.5)
                    tm = smp.tile([D, BT], F32, tag="tm", name="tm")
                    nc.gpsimd.tensor_mul(tm[:], et[:], epsT[:, bs])
                    nc.gpsimd.tensor_add(r(yT[bt][:]), tm[:], mulvT[0:D, bs])

                # (decoder stage defs below; emission is unified with the
                # encoder so decoder work fills encoder-region engine gaps)

                # ---- sequential causal decoder ----
                # Software-pipelined emission: the in-order engine queues
                # preserve emission order, so interleaving stages of
                # consecutive iterations is what lets iteration k+1's L1 run
                # while k's relu/L2 are still in flight.
                NIT = D * NBT
                st1, st2, st3, st4 = {}, {}, {}, {}

                def stage_l1(k):
                    i, b = divmod(k, NBT)
                    ke = i + 1
                    if i == D - 1:
                        # rows 0..D-2 of btile b are final: stream them out now;
                        # row D-1 goes straight from its evac tile (below), so
                        # the tail never waits on a full-tile output DMA.
                        nc.sync.dma_start(out=yT_dr[0:D - 1, ts(b, BT)],
                                          in_=yT[b][0:D - 1, :])
                    p1a = psp.tile([128, BT], F32, tag="ps", name="p1a")
                    nc.tensor.matmul(p1a[:], r(W1sb[0:ke, i, 0:128]),
                                     r(yT[b][0:ke, :]), start=True, stop=True)
                    p1b = psp.tile([128, BT], F32, tag="ps", name="p1b")
                    nc.tensor.matmul(p1b[:], r(W1sb[0:ke, i, 128:256]),
                                     r(yT[b][0:ke, :]), start=True, stop=True)
                    st1[k] = (i, b, p1a, p1b)

                def stage_relu1(k):
                    i, b, p1a, p1b = st1.pop(k)
                    t1a = h1p.tile([128, BT], F32, tag="t1a", name="t1a")
                    nc.scalar.activation(r(t1a[:]), p1a[:], AF.Relu,
                                         bias=b1sb[:, i, 0:1])
                    t1b = h1p.tile([128, BT], F32, tag="t1b", name="t1b")
                    nc.vector.tensor_scalar(r(t1b[:]), p1b[:], b1sb[:, i, 1:2],
                                            0.0, AluOpType.add, AluOpType.max)
                    st2[k] = (i, b, t1a, t1b)

                def stage_l2(k):
                    i, b, t1a, t1b = st2.pop(k)
                    p2a = psp.tile([128, BT], F32, tag="ps", name="p2a")
                    nc.tensor.matmul(p2a[:], r(W2sb[:, i, 0, 0:128]), r(t1a[:]),
                                     start=True, stop=False)
                    nc.tensor.matmul(p2a[:], r(W2sb[:, i, 1, 0:128]), r(t1b[:]),
                                     start=False, stop=True)
                    p2b = psp.tile([128, BT], F32, tag="ps", name="p2b")
                    nc.tensor.matmul(p2b[:], r(W2sb[:, i, 0, 128:256]), r(t1a[:]),
                                     start=True, stop=False)
                    nc.tensor.matmul(p2b[:], r(W2sb[:, i, 1, 128:256]), r(t1b[:]),
                                     start=False, stop=True)
                    st3[k] = (i, b, p2a, p2b)

                def stage_relu2(k):
                    i, b, p2a, p2b = st3.pop(k)
                    # plain fp32 out: consumed by the fp32 flipped L3
                    t2a = h2p.tile([128, BT], F32, tag="t2a", name="t2a")
                    nc.scalar.activation(t2a[:], p2a[:], AF.Relu,
                                         bias=b2sb[:, i, 0:1])
                    t2b = h2p.tile([128, BT], F32, tag="t2b", name="t2b")
                    nc.vector.tensor_scalar(t2b[:], p2b[:], b2sb[:, i, 1:2],
                                            0.0, AluOpType.add, AluOpType.max)
                    st4[k] = (i, b, t2a, t2b)

                def stage_l3(k):
                    i, b, t2a, t2b = st4.pop(k)
                    # flipped: t2 stride-4 chunks stationary, W3 halves moving.
                    # psum (p, c) = y[i] of batch column 4p+c, so the linear
                    # [128,4]->[1,512] DMA lands in batch order.
                    py = psp.tile([128, 4], F32, tag="ps", name="py")
                    for c in range(4):
                        nc.tensor.matmul(py[:, c:c + 1], t2a[:, c::4],
                                         W3sb[:, i, 0:1], start=True, stop=False)
                        nc.tensor.matmul(py[:, c:c + 1], t2b[:, c::4],
                                         W3sb[:, i, 1:2], start=False, stop=True)
                    sb44 = smp.tile([128, 4], F32, tag="srow", name="sb44",
                                    bufs=6)
                    if k % 4 == 1:
                        nc.vector.tensor_scalar(sb44[:], py[:], b3bc[:, i:i + 1],
                                                None, AluOpType.add)
                    else:
                        nc.scalar.activation(sb44[:], py[:], AF.Identity,
                                             bias=b3bc[:, i:i + 1])
                    if i == D - 1:
                        nc.sync.dma_start(out=yT_dr[i:i + 1, ts(b, BT)],
                                          in_=sb44[:])
                    else:
                        nc.sync.dma_start(out=r(yT[b][i:i + 1, :]), in_=r(sb44[:]))

                # 4-deep pipeline: every PE stage consumes only results from
                # strictly earlier steps, so PE never waits on same-step
                # vector work.
                # Unified emission: encoder btile stages (offset by 3 so
                # tail(b) precedes decoder l1(0, b)) interleave with decoder
                # steps; W2 nodes 8..32 stream in as small pieces so the
                # per-node y-row DMAs (critical path) never queue behind a
                # multi-us weight transfer.
                w2_pieces = [slice(8 + j, 9 + j) for j in range(24)]

                for bt in range(NBT + 2):
                    if bt >= 2:
                        enc_tail(bt - 2)
                    if bt < NBT:
                        enc_front(bt)
                    if 1 <= bt < NBT + 1:
                        enc_mid(bt - 1)

                for k in range(NIT + 3):
                    if k % 2 == 0 and w2_pieces:
                        isl = w2_pieces.pop(0)
                        nc.sync.dma_start(
                            out=r(W2sb[:, isl, :, :]),
                            in_=r(W2_d[isl, :, :].rearrange(
                                "i (k p) c -> p i k c", k=2)))
                    if 2 <= k < NIT + 2:
                        stage_relu2(k - 2)
                    if 3 <= k:
                        stage_l3(k - 3)
                    if k < NIT:
                        stage_l1(k)
                        stage_relu1(k)
                    if 1 <= k < NIT + 1:
                        stage_l2(k - 1)


    nc.compile()
    return nc


_NC_CACHE = None


def _get_nc():
    global _NC_CACHE
    if _NC_CACHE is None:
        _NC_CACHE = _make_nc()
    return _NC_CACHE


def _run(inputs, trace=False):
    f32c = lambda a: np.ascontiguousarray(np.asarray(a), dtype=np.float32)
    xorT = f32c(inputs["xor"]).T   # [D, B]
    epsT = f32c(inputs["eps"]).T
    shared = {k: f32c(inputs[k]) for k in
              ["We1", "be1", "We2", "be2", "We3", "be3",
               "W1", "b1", "W2", "b2", "W3"]}
    shared["b3bc"] = np.ascontiguousarray(
        np.broadcast_to(f32c(inputs["b3"])[None, :], (128, D)))
    in_maps = []
    for c in range(NCORES):
        m = dict(shared)
        m["xorT"] = np.ascontiguousarray(xorT[:, c * BL:(c + 1) * BL])
        m["epsT"] = np.ascontiguousarray(epsT[:, c * BL:(c + 1) * BL])
        in_maps.append(m)
    nc = _get_nc()
    res = run_bass_kernel_spmd(nc, in_maps, core_ids=list(range(NCORES)),
                               trace=trace)
    gather = lambda nm: np.ascontiguousarray(
        np.concatenate([r[nm] for r in res.results], axis=1).T)
    y = gather("yT")
    mu = gather("muT")
    lv = gather("lvT")
    return (y, mu, lv, y), res


def kernel(**inputs):
    out, _ = _run(inputs)
    return out
